# revision 4
# baseline (speedup 1.0000x reference)
"""Trainium2 Bass kernel for nn_Block_6725918785547 (dense_cnn encoder block).

Strategy: data-parallel over batch N=16 across 8 NeuronCores (2 images/core).
Each core runs the full block on its 2 images; no collectives.

Per-core pipeline (activations stay in SBUF; layout [C_partition, free]):
  conv_skip (1x1, f32r matmuls) -> pixel-norm (ones-matmul channel reduce +
  K=1 broadcast matmul) -> SiLU (ACT, writes zero-padded 34x34 tiles) ->
  res0 3x3 conv (shifted-window matmuls, PSUM-accumulated) -> SiLU*c ->
  res1 3x3 conv -> residual add -> qkv 1x1 conv emitted TRANSPOSED
  ([pos, ch]) so per-head L2 norms are free-dim reductions -> normalize ->
  PE-transpose q,k back to [ch, pos] (bf16) -> per-head S' = k^T q in
  [kpos, qpos] layout -> exp on ACT -> P@V with an appended ones-column
  (M=65) so softmax row-sums come free -> normalize via K=1 broadcast
  matmul -> proj 1x1 conv -> residual add -> clip.

Weight normalization (weight-norm over fan-in), mp_silu/mp_sum scalar gains
and the tiny emb projection (c = emb @ W_emb^T * gain + 1, a [16,512]
per-channel scale) are folded on the host into the packed weights / scale
vectors; all O(N*C*H*W) compute runs on device.
"""

import numpy as np
import ml_dtypes

import concourse.bass as bass
import concourse.mybir as mybir
import concourse.tile as tile
from concourse import bacc
from concourse.bass_utils import run_bass_kernel_spmd
from concourse.masks import make_identity

P = 128
F32 = mybir.dt.float32
F32R = mybir.dt.float32r
BF16 = mybir.dt.bfloat16

EPS = 1e-4
MP_SILU_C = 0.596
RES_T = 0.3
ATTN_T = 0.3
CLIP = 256.0
HEADS = 8
CH = 64  # head dim

N_CORES = 8
IMG = 2          # images per core
HW = 1024        # 32*32
H = W = 32
PADW = 34        # padded spatial

_DEN_R = float(np.sqrt((1.0 - RES_T) ** 2 + RES_T**2))
C1 = (1.0 - RES_T) / _DEN_R     # residual: x-side gain
C2 = RES_T / _DEN_R             # residual: y-side gain
_DEN_A = float(np.sqrt((1.0 - ATTN_T) ** 2 + ATTN_T**2))
D1 = (1.0 - ATTN_T) / _DEN_A
D2 = ATTN_T / _DEN_A


# ---------------------------------------------------------------- device code

def build_kernel():
    nc = bacc.Bacc("TRN2", target_bir_lowering=False)

    xin_d = nc.dram_tensor("xin", [P, 2, IMG, HW], F32R, kind="ExternalInput")
    wskip_d = nc.dram_tensor("wskip", [P, 2, 512], F32R, kind="ExternalInput")
    w0_d = nc.dram_tensor("w0", [4, P, 4, 9, P], F32R, kind="ExternalInput")
    w1_d = nc.dram_tensor("w1", [4, P, 4, 9, P], F32R, kind="ExternalInput")
    wqkv_d = nc.dram_tensor("wqkv", [P, 4, 1536], F32R, kind="ExternalInput")
    wproj_d = nc.dram_tensor("wproj", [P, 4, 512], BF16, kind="ExternalInput")
    cvec_d = nc.dram_tensor("cvec", [P, 4, IMG], F32, kind="ExternalInput")
    ones_d = nc.dram_tensor("ones", [P, P], F32R, kind="ExternalInput")
    out_d = nc.dram_tensor("out", [P, 4, IMG, HW], F32, kind="ExternalOutput")

    with (
        nc.allow_low_precision(reason="deliberate f32r compute pipeline"),
        tile.TileContext(nc) as tc,
    ):
        with tc.tile_pool(name="persist", bufs=1) as pp:
            # persistent tiles
            x2 = pp.tile([P, 4, IMG, HW], F32R)
            wproj = pp.tile([P, 4, 512], BF16)
            cvec = pp.tile([P, 4, IMG], F32)
            onesT = pp.tile([P, P], F32R)
            ident = pp.tile([P, P], BF16)

            nc.sync.dma_start(wproj[:], wproj_d[:])
            nc.sync.dma_start(cvec[:], cvec_d[:])
            nc.sync.dma_start(onesT[:], ones_d[:])
            make_identity(nc, ident)

            # ---------------- trunk: conv_skip, pixel norm, res convs -------
            with (
                tc.tile_pool(name="trunk", bufs=1) as tp,
                tc.tile_pool(name="tpsum", bufs=1, space="PSUM") as tps,
            ):
                wskip = tp.tile([P, 2, 512], F32R)
                nc.sync.dma_start(wskip[:], wskip_d[:])

                for img in range(IMG):
                    xin = tp.tile([P, 2, HW], F32R, tag="xin", bufs=2, name="xin")
                    nc.sync.dma_start(xin[:], xin_d[:, :, img, :])
                    x1 = tp.tile([P, 4, HW], F32R, tag="x1", bufs=2, name="x1")
                    xpad0 = tp.tile([P, 4, PADW, PADW], F32R, tag="xpad",
                                    bufs=2, name="xpad0")
                    xpad1 = tp.tile([P, 4, PADW, PADW], F32R, tag="xpad",
                                    bufs=2, name="xpad1")
                    nc.gpsimd.memset(xpad0[:].bitcast(F32), 0.0)
                    nc.gpsimd.memset(xpad1[:].bitcast(F32), 0.0)

                    # --- stage A: conv_skip (1x1, 256 -> 512) ---
                    for coc in range(4):
                        for nt in range(2):
                            ps = tps.tile([P, 512], F32, tag="conv", bufs=4,
                                          name="ps_skip")
                            for cic in range(2):
                                nc.tensor.matmul(
                                    ps[:],
                                    wskip[:, cic, coc * P : (coc + 1) * P],
                                    xin[:, cic, nt * 512 : (nt + 1) * 512],
                                    start=(cic == 0),
                                    stop=(cic == 1),
                                )
                            nc.any.tensor_copy(
                                x1[:, coc, nt * 512 : (nt + 1) * 512], ps[:]
                            )

                    # --- stage B: pixel norm over channels ---
                    for nt in range(2):
                        ss = tps.tile([P, 512], F32, tag="ss", bufs=2, name="ps_ss")
                        for coc in range(4):
                            sq = tp.tile([P, 512], F32R, tag="sq", bufs=2, name="sq")
                            nc.vector.tensor_mul(
                                sq[:],
                                x1[:, coc, nt * 512 : (nt + 1) * 512],
                                x1[:, coc, nt * 512 : (nt + 1) * 512],
                            )
                            nc.tensor.matmul(
                                ss[0:1, :], onesT[:, 0:1], sq[:],
                                start=(coc == 0), stop=(coc == 3),
                            )
                        rs = tp.tile([P, 512], F32R, tag="rs", bufs=2, name="rs")
                        # rs = sqrt(ss/512); rs = 1/((rs+eps)/(D1*C1))
                        nc.scalar.activation(
                            rs[0:1, :], ss[0:1, :],
                            mybir.ActivationFunctionType.Sqrt, scale=1.0 / 512.0,
                        )
                        nc.vector.tensor_scalar(
                            rs[0:1, :], rs[0:1, :], EPS, 1.0 / (D1 * C1),
                            mybir.AluOpType.add, mybir.AluOpType.mult,
                        )
                        nc.vector.reciprocal(rs[0:1, :], rs[0:1, :])
                        rb = tps.tile([P, 512], F32, tag="rb", bufs=2, name="ps_rb")
                        nc.tensor.matmul(rb[:], onesT[0:1, :], rs[0:1, :],
                                         start=True, stop=True)
                        for coc in range(4):
                            nc.vector.tensor_mul(
                                x1[:, coc, nt * 512 : (nt + 1) * 512],
                                x1[:, coc, nt * 512 : (nt + 1) * 512],
                                rb[:],
                            )
                    # x1 now holds xn_s = D1*C1*normalize(conv_skip(x))

                    # --- stage C: silu -> xpad0 (res0 input) ---
                    for coc in range(4):
                        nc.scalar.activation(
                            xpad0[:, coc, 1:33, 1:33],
                            x1[:, coc, :].rearrange("p (h w) -> p h w", h=H),
                            mybir.ActivationFunctionType.Silu,
                            scale=1.0 / (D1 * C1),
                        )

                    # --- stage D: res0; evict = silu(c*y) -> xpad1 ---
                    for coc in range(4):
                        w0c = tp.tile([P, 4, 9, P], F32R, tag="wres", bufs=2,
                                      name="w0c")
                        nc.sync.dma_start(w0c[:], w0_d[coc])
                        for half in range(2):
                            h0 = half * 16
                            ps = tps.tile([P, 512], F32, tag="conv", bufs=4,
                                          name="ps_r0")
                            first = True
                            for cic in range(4):
                                for ky in range(3):
                                    for kx in range(3):
                                        nc.tensor.matmul(
                                            ps[:],
                                            w0c[:, cic, ky * 3 + kx, :],
                                            xpad0[:, cic, h0 + ky : h0 + ky + 16,
                                                  kx : kx + 32],
                                            start=first,
                                            stop=(cic == 3 and ky == 2 and kx == 2),
                                        )
                                        first = False
                            nc.scalar.activation(
                                xpad1[:, coc, 1 + h0 : 17 + h0, 1:33],
                                ps[:].rearrange("p (h w) -> p h w", h=16),
                                mybir.ActivationFunctionType.Silu,
                                scale=cvec[:, coc, img, None],
                            )

                    # --- stage E: res1; evict = psum + xn_s -> x2 ---
                    for coc in range(4):
                        w1c = tp.tile([P, 4, 9, P], F32R, tag="wres", bufs=2,
                                      name="w1c")
                        nc.sync.dma_start(w1c[:], w1_d[coc])
                        for half in range(2):
                            h0 = half * 16
                            ps = tps.tile([P, 512], F32, tag="conv", bufs=4,
                                          name="ps_r1")
                            first = True
                            for cic in range(4):
                                for ky in range(3):
                                    for kx in range(3):
                                        nc.tensor.matmul(
                                            ps[:],
                                            w1c[:, cic, ky * 3 + kx, :],
                                            xpad1[:, cic, h0 + ky : h0 + ky + 16,
                                                  kx : kx + 32],
                                            start=first,
                                            stop=(cic == 3 and ky == 2 and kx == 2),
                                        )
                                        first = False
                            sl = slice(half * 512, half * 512 + 512)
                            nc.vector.tensor_add(
                                x2[:, coc, img, sl], ps[:], x1[:, coc, sl]
                            )

            # ---------------- attention ------------------------------------
            with (
                tc.tile_pool(name="attn", bufs=1) as ap,
                tc.tile_pool(name="apsum", bufs=1, space="PSUM") as aps,
            ):
                x3 = ap.tile([P, 4, IMG, HW], F32)
                wqkv = ap.tile([P, 4, 1536], F32R)
                nc.sync.dma_start(wqkv[:], wqkv_d[:])

                for img in range(IMG):
                    # qkv conv, transposed out: qkvT[pos, col], col = s*512+h*64+c
                    qkvT = ap.tile([P, 8, 1536], BF16, tag="qkvT", name="qkvT")
                    for pc in range(8):
                        psb = aps.tile([P, 1024], F32, tag="big", bufs=2,
                                       name="ps_qkv")
                        pss = aps.tile([P, 512], F32, tag="small", bufs=3,
                                       name="ps_qkv2")
                        for ncol in range(3):
                            ps = (
                                psb[:, (ncol % 2) * 512 : (ncol % 2) * 512 + 512]
                                if ncol < 2
                                else pss[:]
                            )
                            for cic in range(4):
                                nc.tensor.matmul(
                                    ps,
                                    x2[:, cic, img, pc * P : (pc + 1) * P],
                                    wqkv[:, cic, ncol * 512 : (ncol + 1) * 512],
                                    start=(cic == 0),
                                    stop=(cic == 3),
                                )
                        nc.any.tensor_copy(qkvT[:, pc, 0:1024], psb[:])
                        nc.any.tensor_copy(qkvT[:, pc, 1024:1536], pss[:])

                    # per-(pos, s, h) L2 norms over the 64 head channels
                    nrm = ap.tile([P, 8, 24], F32, tag="nrm", name="nrm")
                    for pc in range(8):
                        sqv = ap.tile([P, 1536], F32, tag="sqv", bufs=2, name="sqv")
                        nc.scalar.activation(
                            sqv[:], qkvT[:, pc, :],
                            mybir.ActivationFunctionType.Square,
                        )
                        nc.vector.tensor_reduce(
                            nrm[:, pc, :],
                            sqv[:].rearrange("p (s c) -> p s c", c=CH),
                            axis=mybir.AxisListType.X,
                            op=mybir.AluOpType.add,
                        )
                    # r = 1/(eps + sqrt(nrm/64))
                    nc.scalar.activation(
                        nrm[:], nrm[:], mybir.ActivationFunctionType.Sqrt,
                        scale=1.0 / CH,
                    )
                    nc.vector.tensor_scalar_add(nrm[:], nrm[:], EPS)
                    nc.vector.reciprocal(nrm[:], nrm[:])

                    # normalize q,k in place; v -> vT_aug with ones column
                    vT = ap.tile([P, 8, HEADS, CH + 1], BF16, tag="vT", name="vT")
                    nc.vector.memset(vT[:, :, :, CH], 1.0)
                    qkvT4 = qkvT[:].rearrange("p k (s c) -> p k s c", c=CH)
                    nc.vector.tensor_mul(
                        qkvT4[:, :, 0:16, :],
                        qkvT4[:, :, 0:16, :],
                        nrm[:, :, 0:16, None].to_broadcast([P, 8, 16, CH]),
                    )
                    nc.vector.tensor_mul(
                        vT[:, :, :, 0:CH],
                        qkvT4[:, :, 16:24, :],
                        nrm[:, :, 16:24, None].to_broadcast([P, 8, HEADS, CH]),
                    )

                    # PE-transpose q,k back to [ch, pos] (bf16)
                    qh = ap.tile([P, 4, HW], BF16, tag="qh", name="qh")
                    kh = ap.tile([P, 4, HW], BF16, tag="kh", name="kh")
                    for tgt, off in ((qh, 0), (kh, 512)):
                        for hc in range(4):
                            for pg in range(2):
                                pst = aps.tile([P, 512], BF16, tag="small", bufs=3,
                                               name="ps_tp")
                                for i in range(4):
                                    pc = pg * 4 + i
                                    nc.tensor.transpose(
                                        pst[:, i * P : (i + 1) * P],
                                        qkvT[:, pc, off + hc * P : off + (hc + 1) * P],
                                        ident[:],
                                    )
                                nc.vector.tensor_copy(
                                    tgt[:, hc, pg * 512 : (pg + 1) * 512], pst[:]
                                )

                    # attention per (head, q-tile)
                    oall = ap.tile([P, 4, HW], BF16, tag="oall", name="oall")
                    for h in range(HEADS):
                        hp = (h % 2) * CH
                        hc = h // 2
                        for qt in range(2):
                            pq = ap.tile([P, 8, 512], BF16, tag="pq", bufs=2,
                                         name="pq")
                            for g in range(4):
                                psb = aps.tile([P, 1024], F32, tag="big", bufs=2,
                                               name="ps_s")
                                for i in range(2):
                                    kc = g * 2 + i
                                    nc.tensor.matmul(
                                        psb[:, i * 512 : (i + 1) * 512],
                                        kh[hp : hp + CH, hc, kc * P : (kc + 1) * P],
                                        qh[hp : hp + CH, hc,
                                           qt * 512 : (qt + 1) * 512],
                                        start=True, stop=True,
                                    )
                                nc.scalar.activation(
                                    pq[:, 2 * g : 2 * g + 2, :], psb[:],
                                    mybir.ActivationFunctionType.Exp,
                                    scale=1.0 / 8.0,
                                )
                            pso = aps.tile([P, 512], F32, tag="small", bufs=3,
                                           name="ps_o")
                            for kc in range(8):
                                nc.tensor.matmul(
                                    pso[0 : CH + 1, :],
                                    vT[:, kc, h, :],
                                    pq[:, kc, :],
                                    start=(kc == 0),
                                    stop=(kc == 7),
                                )
                            otmp = ap.tile([P, 512], F32, tag="otmp", bufs=2,
                                           name="otmp")
                            nc.scalar.copy(otmp[0 : CH + 1, :], pso[0 : CH + 1, :])
                            rr = ap.tile([P, 512], F32R, tag="rr", bufs=2, name="rr")
                            nc.vector.reciprocal(rr[0:1, :], otmp[CH : CH + 1, :])
                            psr = aps.tile([P, 512], F32, tag="small", bufs=3,
                                           name="ps_r")
                            nc.tensor.matmul(
                                psr[0:CH, :], onesT[0:1, 0:CH], rr[0:1, :],
                                start=True, stop=True,
                            )
                            nc.vector.tensor_mul(
                                oall[hp : hp + CH, hc, qt * 512 : (qt + 1) * 512],
                                otmp[0:CH, :],
                                psr[0:CH, :],
                            )

                    # proj + residual
                    for coc in range(4):
                        for nt in range(2):
                            ps = aps.tile([P, 512], F32, tag="small", bufs=3,
                                          name="ps_p")
                            for cic in range(4):
                                nc.tensor.matmul(
                                    ps[:],
                                    wproj[:, cic, coc * P : (coc + 1) * P],
                                    oall[:, cic, nt * 512 : (nt + 1) * 512],
                                    start=(cic == 0),
                                    stop=(cic == 3),
                                )
                            sl = slice(nt * 512, nt * 512 + 512)
                            nc.vector.tensor_add(
                                x3[:, coc, img, sl], ps[:],
                                x2[:, coc, img, sl].bitcast(F32),
                            )

                # clip and store
                for coc in range(4):
                    nc.vector.tensor_scalar(
                        x3[:, coc, :, :], x3[:, coc, :, :], CLIP, -CLIP,
                        mybir.AluOpType.min, mybir.AluOpType.max,
                    )
                nc.sync.dma_start(out_d[:], x3[:])

    nc.compile()
    return nc


# ---------------------------------------------------------------- host side

def _normalize_w(w):
    w = w.astype(np.float64)
    axes = tuple(range(1, w.ndim))
    norm = np.sqrt((w**2).sum(axis=axes, keepdims=True))
    alpha = np.sqrt(norm.size / w.size)
    return w / (EPS + alpha * norm)


def _pack_weights(w_skip, w_res0, w_res1, w_emb, w_qkv, w_proj, emb_gain, emb):
    # conv_skip: fan=256, gain=1
    ws = _normalize_w(w_skip[:, :, 0, 0]) / np.sqrt(256.0)
    wskip = ws.T.reshape(2, P, 512).transpose(1, 0, 2)  # [128, 2, 512]

    # res0: fold 1/MP_SILU_C
    w0 = _normalize_w(w_res0.reshape(512, -1)) / np.sqrt(512 * 9.0) / MP_SILU_C
    w0p = (
        w0.reshape(4, P, 4, P, 9)
        .transpose(0, 3, 2, 4, 1)
        .reshape(4, P, 4, 9, P)
    )
    # res1: fold D1*C2/MP_SILU_C
    w1 = (
        _normalize_w(w_res1.reshape(512, -1))
        / np.sqrt(512 * 9.0)
        * (D1 * C2 / MP_SILU_C)
    )
    w1p = (
        w1.reshape(4, P, 4, P, 9)
        .transpose(0, 3, 2, 4, 1)
        .reshape(4, P, 4, 9, P)
    )

    # qkv: reorder rows to [s, h, c]
    wq = _normalize_w(w_qkv[:, :, 0, 0]) / np.sqrt(512.0)  # [1536, 512]
    s_idx, h_idx, c_idx = np.meshgrid(
        np.arange(3), np.arange(HEADS), np.arange(CH), indexing="ij"
    )
    perm = ((h_idx * CH + c_idx) * 3 + s_idx).reshape(-1)
    wqp = wq[perm]  # rows ordered s*512 + h*64 + c
    wqkvT = wqp.T.reshape(4, P, 1536).transpose(1, 0, 2)  # [128, 4, 1536]

    # proj: fold D2
    wp = _normalize_w(w_proj[:, :, 0, 0]) / np.sqrt(512.0) * D2
    wprojT = wp.T.reshape(4, P, 512).transpose(1, 0, 2)  # [128, 4, 512]

    # emb scale: c = emb @ w_emb_n.T * gain + 1  -> [16, 512]
    we = _normalize_w(w_emb) * (float(emb_gain) / np.sqrt(1024.0))
    c = emb.astype(np.float64) @ we.T + 1.0  # [16, 512]

    return (
        np.ascontiguousarray(wskip).astype(np.float32),
        np.ascontiguousarray(w0p).astype(np.float32),
        np.ascontiguousarray(w1p).astype(np.float32),
        np.ascontiguousarray(wqkvT).astype(np.float32),
        np.ascontiguousarray(wprojT).astype(ml_dtypes.bfloat16),
        np.ascontiguousarray(c).astype(np.float32),
    )


_NC_CACHE = None


def kernel(x, emb, w_skip, w_res0, w_res1, w_emb, w_qkv, w_proj, emb_gain):
    global _NC_CACHE
    if _NC_CACHE is None:
        _NC_CACHE = build_kernel()
    nc = _NC_CACHE

    x = np.asarray(x, dtype=np.float32)
    wskip, w0p, w1p, wqkvT, wprojT, c = _pack_weights(
        np.asarray(w_skip, np.float32),
        np.asarray(w_res0, np.float32),
        np.asarray(w_res1, np.float32),
        np.asarray(w_emb, np.float32),
        np.asarray(w_qkv, np.float32),
        np.asarray(w_proj, np.float32),
        np.asarray(emb_gain, np.float32),
        np.asarray(emb, np.float32),
    )
    ones = np.ones((P, P), dtype=np.float32)

    in_maps = []
    for core in range(N_CORES):
        xi = x[core * IMG : (core + 1) * IMG].reshape(IMG, 2, P, HW)
        xi = np.ascontiguousarray(xi.transpose(2, 1, 0, 3))  # [128, 2, IMG, HW]
        ci = c[core * IMG : (core + 1) * IMG]  # [IMG, 512]
        ci = np.ascontiguousarray(ci.T.reshape(4, P, IMG).transpose(1, 0, 2))
        in_maps.append(
            {
                "xin": xi,
                "wskip": wskip,
                "w0": w0p,
                "w1": w1p,
                "wqkv": wqkvT,
                "wproj": wprojT,
                "cvec": ci,
                "ones": ones,
            }
        )

    res = run_bass_kernel_spmd(nc, in_maps, core_ids=list(range(N_CORES)))
    outs = []
    for core in range(N_CORES):
        o = res.results[core]["out"]  # [128, 4, IMG, HW]
        o = o.transpose(2, 1, 0, 3).reshape(IMG, 512, H, W)
        outs.append(o)
    return np.concatenate(outs, axis=0).astype(np.float32)
